# revision 49
# baseline (speedup 1.0000x reference)
"""MixHopNet Trainium2 kernel: 8-core SPMD, node-sharded.

Math (matches the jax reference):
  dinv = rsqrt(deg) with self loops; Ahat = D (A+I) D  (D = diag(dinv))
  h  = relu([x W0, (Ahat x) W1, (Ahat^2 x) W2] + b1)
  y  = [h V0, Ahat (h V1)] + b2
  out = log_softmax(y, axis=1)

Implementation notes:
  - norm is separable (dinv[row]*dinv[col]) -> propagation is a 0/1 adjacency
    sum over pre-scaled features; self loops are fused scale-adds at PSUM evac.
  - gather: gpsimd.dma_gather (int16 idxs -> 32k-row windows), fp16 rows 256B.
    Descriptor generation is the bottleneck engine, so gathers are batched
    (16 chunks = 2048 idxs per call, bounded by the 256-desc/engine ring) and
    spread round-robin over all 4 SWDGE queues (each queue = its own Q7 core
    pair) for 4x parallel descriptor generation.
  - scatter: one-hot matmul on TensorE. S[128 edges, 256 dests] generated on
    VectorE via is_equal(dlocal, iota256); PSUM out is feature-major
    [128 feat, 256 dest] accumulated over all chunks of a 256-dest cell.
  - one program for all 8 cores: per-(cell, window) chunk counts are
    equalized to the cross-core max (pad slots gather row 0 and have
    dlocal=-1 so they contribute nothing).
  - all-gathers of the propagated features between steps (ncfw collectives).
"""

from contextlib import ExitStack

import numpy as np
import ml_dtypes

import concourse.bass as bass
import concourse.mybir as mybir
import concourse.tile as tile
from concourse import bacc, bass_utils

FP16 = mybir.dt.float16
F32 = mybir.dt.float32
I16 = mybir.dt.int16

N_CORES = 8
F_IN, HID, NCLS = 128, 512, 40
WIN = 32768
NBLK = 512
DW = 256                 # dest-cell width (one-hot width, matmul N)
GRP = 8                  # cells per slab group (8*256 = 2048 dests)
# AllGather pieces, in units of slab groups. A single piece is best: each
# collective pays a ~300us fixed cost and its DMAs contend with the gathers.
PIECE_GROUPS = [11]
# chunks per dma_gather call: 8 chunks = 1024 idxs = 65 descs/engine/side.
# Anything larger overflows the SWDGE descriptor ring and hangs the device
# (12-chunk = 97 descs already hangs, so the ring is ~64-96 descs/engine).
CALL_CHUNKS = 8
NQ = 4                   # SWDGE queues (4 Q7 core pairs generate descriptors)


def make_dims(n):
    sl = -(-n // (N_CORES * 512)) * 512       # per-core slice, multiple of 512
    npad = sl * N_CORES
    nsc = sl // DW
    nsg = -(-nsc // GRP)
    assert sum(PIECE_GROUPS) == nsg
    # piece tables (for chunked AllGathers + piece-major source-node space)
    piece_cells = []
    g0 = 0
    for pg in PIECE_GROUPS:
        piece_cells.append(sum(min(GRP, nsc - (g0 + k) * GRP) for k in range(pg)))
        g0 += pg
    piece_rows = [pc * DW for pc in piece_cells]           # per-core rows/piece
    piece_row_start = np.concatenate([[0], np.cumsum(piece_rows)[:-1]]).astype(np.int64)
    piece_base_g = np.concatenate([[0], np.cumsum([N_CORES * r for r in piece_rows])[:-1]]).astype(np.int64)
    return dict(
        N=n, SL=sl, NP=npad,
        NT=sl // 128,            # 128-row dest tiles per core
        NSC=nsc,                 # 256-dest cells per core
        NSG=nsg,                 # slab groups per core
        NW=-(-npad // WIN),      # source windows
        NB=sl // NBLK,           # GEMM n-blocks
        piece_rows=piece_rows, piece_row_start=piece_row_start,
        piece_base_g=piece_base_g,
    )


def _remap_rows(n, dims):
    """Global node id -> row in the piece-major gathered layout."""
    SL, NSC = dims["SL"], dims["NSC"]
    piece_of_group = np.repeat(np.arange(len(PIECE_GROUPS)), PIECE_GROUPS)
    piece_of_cell = piece_of_group[np.arange(NSC) // GRP]
    c = n // SL
    i = n % SL
    p = piece_of_cell[i // DW]
    prow = np.asarray(dims["piece_rows"], np.int64)
    return dims["piece_base_g"][p] + c * prow[p] + (i - dims["piece_row_start"][p])


# ================================================================ host prep

def _wrap16(seg):
    return np.ascontiguousarray(seg.reshape(-1, 16).T)


def host_prep(x, edge_index, w1, b1, w2, b2, dims):
    N, SL, NP, NSC, NW = dims["N"], dims["SL"], dims["NP"], dims["NSC"], dims["NW"]
    NT = dims["NT"]
    x = np.asarray(x, np.float32)
    ei = np.asarray(edge_index)
    row, col = ei[0].astype(np.int64), ei[1].astype(np.int64)

    deg = np.bincount(col, minlength=N).astype(np.float32) + 1.0
    dinv = (1.0 / np.sqrt(deg)).astype(np.float32)
    dinv_p = np.zeros(NP, np.float32)
    dinv_p[:N] = dinv

    # xp lives in the piece-major layout that the chunked AllGathers produce,
    # so all three propagations share one source-row space (and one chunk
    # structure). remap is a permutation of [0, NP).
    remap = _remap_rows(np.arange(NP, dtype=np.int64), dims)
    xp = np.zeros((NP, F_IN), np.float16)
    xp[remap[:N]] = (dinv[:, None] * x).astype(np.float16)

    w1 = np.asarray(w1, np.float32)
    wcat = np.concatenate([w1[0], w1[1], w1[2]], axis=1).astype(np.float16)
    w2 = np.asarray(w2, np.float32)
    vcat = np.concatenate([w2[0], w2[1]], axis=1)  # [1536, 80]
    vt = np.ascontiguousarray(
        vcat.reshape(12, 128, 80).transpose(1, 0, 2).reshape(128, 12 * 80)
    ).astype(np.float16)
    b1w = np.ascontiguousarray(np.asarray(b1, np.float32).reshape(12, 128).T)
    b2 = np.asarray(b2, np.float32)
    b2arep = np.tile(b2[:40][None, :], (128, 1)).astype(np.float32)
    b2brep = np.tile(b2[40:][None, :], (128, 1)).astype(np.float32)

    ident = np.eye(128, dtype=np.float16)
    ident32 = np.eye(128, dtype=np.float32)
    ones32 = np.ones((1, 128), np.float32)

    keep = row != col
    row, col = row[keep], col[keep]

    row = remap[row]     # edge sources live in the piece-major space

    # ---- per-core edge cells, then cross-core-equalized chunk structure
    cores = []
    for c in range(N_CORES):
        lo = c * SL
        m = (col >= lo) & (col < lo + SL)
        r, d = row[m], col[m] - lo
        cell = d // DW
        w = r // WIN
        order = np.lexsort((r, w, cell))
        r, d, cell, w = r[order], d[order], cell[order], w[order]
        key = cell * NW + w
        counts = np.bincount(key, minlength=NSC * NW)
        starts = np.zeros(NSC * NW + 1, np.int64)
        starts[1:] = np.cumsum(counts)
        cores.append(dict(r=r, d=d, counts=counts, starts=starts))

    all_counts = np.stack([cr["counts"] for cr in cores])          # [8, NSC*NW]
    cell_chunks = np.max(-(-all_counts // 128), axis=0)            # shared
    # chunk axis order: (sg, w, cell-within-sg, chunk)
    NSG = -(-NSC // GRP)
    chunk_off = {}
    slab = {}      # (sg, w) -> (chunk_base, n_chunks)
    ctot = 0
    for sg in range(NSG):
        cls = range(sg * GRP, min(sg * GRP + GRP, NSC))
        for w in range(NW):
            base = ctot
            for cl in cls:
                chunk_off[(cl, w)] = ctot
                ctot += int(cell_chunks[cl * NW + w])
            slab[(sg, w)] = (base, ctot - base)

    struct = dict(NSG=NSG, CTOT=ctot, cell_chunks=cell_chunks,
                  chunk_off=chunk_off, slab=slab)

    # iota replicated to the max chunks-per-cell so S-gen's in1 is contiguous
    mxnc = int(cell_chunks.max())
    struct["MXNC"] = mxnc
    iota = np.tile(np.arange(DW, dtype=np.float16)[None, :], (128, mxnc))

    # ---- per-core idx / dlocal arrays in the shared layout
    per_core = []
    for c in range(N_CORES):
        cr = cores[c]
        idx_all = np.zeros((16, ctot * 8), np.int16)
        dl_all = np.full((128, ctot), -1.0, np.float16)
        for cl in range(NSC):
            for w in range(NW):
                k = cl * NW + w
                n = int(cr["counts"][k])
                if n == 0:
                    continue
                co = chunk_off[(cl, w)]
                npad = int(cell_chunks[k]) * 128
                a = cr["starts"][k]
                iseg = np.zeros(npad, np.int16)
                iseg[:n] = (cr["r"][a:a + n] - w * WIN).astype(np.int16)
                dseg = np.full(npad, -1.0, np.float16)
                dseg[:n] = (cr["d"][a:a + n] - cl * DW).astype(np.float16)
                idx_all[:, co * 8:co * 8 + npad // 16] = _wrap16(iseg)
                dl_all[:, co:co + npad // 128] = dseg.reshape(-1, 128).T
        per_core.append(dict(idx=np.tile(idx_all, (8, 1)), dl=dl_all))

    # ---- per-core dense inputs
    for c in range(N_CORES):
        lo = c * SL
        hi = min(lo + SL, N)
        nr = hi - lo
        xT = np.zeros((128, SL), np.float16)
        xppT = np.zeros((128, SL), np.float16)
        if nr > 0:
            xT[:, :nr] = x[lo:hi].T.astype(np.float16)
            xppT[:, :nr] = (dinv[lo:hi][None, :] ** 3 * x[lo:hi].T).astype(np.float16)
        dv = dinv_p[lo:lo + SL]
        per_core[c].update(
            xT=xT, xppT=xppT,
            dinvrow=dv.reshape(1, SL).astype(np.float32),
            dinv4w=np.ascontiguousarray(dv.reshape(NT, 128).T.astype(np.float32)),
        )

    shared = dict(xp=xp, wcat=wcat, vt=vt, b1w=b1w, b2arep=b2arep, b2brep=b2brep,
                  iota=iota, ident=ident, ident32=ident32, ones32=ones32)
    return shared, per_core, struct


# ================================================================ builder

def build(dims, struct):
    SL, NP, NW, NSC, NT, NB = (dims[k] for k in ("SL", "NP", "NW", "NSC", "NT", "NB"))
    piece_rows = dims["piece_rows"]
    piece_row_start = dims["piece_row_start"]
    piece_base_g = dims["piece_base_g"]
    piece_end_group = np.cumsum(PIECE_GROUPS) - 1      # last group idx per piece
    NSG, CTOT = struct["NSG"], struct["CTOT"]
    cell_chunks, chunk_off, slab = (struct[k] for k in ("cell_chunks", "chunk_off", "slab"))

    nc = bacc.Bacc("TRN2", target_bir_lowering=False, debug=False,
                   num_devices=N_CORES, num_swdge_queues=NQ)

    # DRAM tensors
    xp_d = nc.dram_tensor("xp", [NP, F_IN], FP16, kind="ExternalInput")
    idx_d = nc.dram_tensor("idx", [128, CTOT * 8], I16, kind="ExternalInput")
    dl_d = nc.dram_tensor("dl", [128, CTOT], FP16, kind="ExternalInput")
    xT_d = nc.dram_tensor("xT", [128, SL], FP16, kind="ExternalInput")
    xppT_d = nc.dram_tensor("xppT", [128, SL], FP16, kind="ExternalInput")
    dinvrow_d = nc.dram_tensor("dinvrow", [1, SL], F32, kind="ExternalInput")
    dinv4w_d = nc.dram_tensor("dinv4w", [128, NT], F32, kind="ExternalInput")
    wcat_d = nc.dram_tensor("wcat", [128, 3 * HID], FP16, kind="ExternalInput")
    vt_d = nc.dram_tensor("vt", [128, 12 * 80], FP16, kind="ExternalInput")
    b1w_d = nc.dram_tensor("b1w", [128, 12], F32, kind="ExternalInput")
    b2arep_d = nc.dram_tensor("b2arep", [128, 40], F32, kind="ExternalInput")
    b2brep_d = nc.dram_tensor("b2brep", [128, 40], F32, kind="ExternalInput")
    MXNC = struct["MXNC"]
    iota_d = nc.dram_tensor("iota", [128, MXNC * DW], FP16, kind="ExternalInput")
    ident_d = nc.dram_tensor("ident", [128, 128], FP16, kind="ExternalInput")
    ident32_d = nc.dram_tensor("ident32", [128, 128], F32, kind="ExternalInput")
    ones32_d = nc.dram_tensor("ones32", [1, 128], F32, kind="ExternalInput")
    out_d = nc.dram_tensor("out", [SL, 80], F32, kind="ExternalOutput")

    cc_in1 = nc.dram_tensor("cc_in1", [SL, F_IN], FP16)
    cc_out1 = nc.dram_tensor("cc_out1", [NP, F_IN], FP16, addr_space="Shared")
    cc_in2 = nc.dram_tensor("cc_in2", [SL, 128], FP16)
    cc_out2 = nc.dram_tensor("cc_out2", [NP, 128], FP16, addr_space="Shared")
    z0n_d = nc.dram_tensor("z0n", [SL, 40], F32)
    x1T_d = nc.dram_tensor("x1Tspill", [128, SL], FP16)
    x2T_d = nc.dram_tensor("x2Tspill", [128, SL], FP16)

    win_rows = [min(WIN, NP - w * WIN) for w in range(NW)]
    qctr = [0]                 # round-robin SWDGE queue assignment

    with tile.TileContext(nc) as tc, ExitStack() as ctx:
        cpool = ctx.enter_context(tc.tile_pool(name="consts", bufs=1))
        gpool = ctx.enter_context(tc.tile_pool(name="gslab", bufs=8))
        spool = ctx.enter_context(tc.tile_pool(name="sslab", bufs=4))
        ipool = ctx.enter_context(tc.tile_pool(name="idxs", bufs=8))
        epool = ctx.enter_context(tc.tile_pool(name="evac", bufs=2))
        hpool = ctx.enter_context(tc.tile_pool(name="hblk", bufs=2))
        ppool = ctx.enter_context(tc.tile_pool(name="psum", bufs=4, space="PSUM"))
        tpool = ctx.enter_context(tc.tile_pool(name="psum_t", bufs=2, space="PSUM"))

        # ---- constants / persistent slabs
        def load(shape, dt, src, nm):
            t = cpool.tile(shape, dt, tag=nm, name=nm)
            nc.sync.dma_start(out=t[:], in_=src[:])
            return t

        dl_sb = load([128, CTOT], FP16, dl_d, "c_dl")
        dinv4w = load([128, NT], F32, dinv4w_d, "c_dinv4w")
        wcat = load([128, 3 * HID], FP16, wcat_d, "c_wcat")
        vt = load([128, 12 * 80], FP16, vt_d, "c_vt")
        b1w = load([128, 12], F32, b1w_d, "c_b1w")
        b2arep = load([128, 40], F32, b2arep_d, "c_b2arep")
        b2brep = load([128, 40], F32, b2brep_d, "c_b2brep")
        iota = cpool.tile([128, MXNC, DW], FP16, tag="c_iota", name="c_iota")
        nc.sync.dma_start(out=iota[:],
                          in_=iota_d[:].rearrange("p (a b) -> p a b", b=DW))
        ident = load([128, 128], FP16, ident_d, "c_ident")
        ident32 = load([128, 128], F32, ident32_d, "c_ident32")
        ones32 = load([1, 128], F32, ones32_d, "c_ones32")

        def repl_row(row_dram, c0, w):
            rowwin = epool.tile([1, w], F32, tag="rowwin", name="rowwin")
            nc.sync.dma_start(out=rowwin[:], in_=row_dram[:, c0:c0 + w])
            r = tpool.tile([128, w], F32, tag="tp", name="replrow")
            nc.tensor.matmul(out=r[:], lhsT=ones32[:], rhs=rowwin[:], start=True, stop=True)
            return r

        def prop(src_dram, elem, lhs_feats, evac_cell, after_group=None):
            """One propagation: gathers + one-hot scatter matmuls, then per-cell
            evacuation. src rows are [elem] fp16 (256B). evac_cell(cl, acc) with
            acc = PSUM [lhs_feats, DW] f32 accumulated A-sum (feature-major)."""
            for sg in range(NSG):
                cls = list(range(sg * GRP, min(sg * GRP + GRP, NSC)))
                # Two cells share one PSUM bank ([128, 2, DW] f32 = 2 KiB).
                # A matmul with start=True clears has_written for the WHOLE
                # bank, so the pair forms ONE accumulation group: only the
                # pair's first matmul sets start, only its last sets stop
                # (flags=0 overwrites-where-unwritten, which acts as the
                # second cell's start).
                pair_of = {cl: j // 2 for j, cl in enumerate(cls)}
                npairs = (len(cls) + 1) // 2
                pairs = {p: ppool.tile([128, 2, DW], F32, tag="acc",
                                       name=f"accp{sg}_{p}") for p in range(npairs)}
                accs = {cl: pairs[pair_of[cl]][:, j % 2, :] for j, cl in enumerate(cls)}
                remaining = {p: 0 for p in range(npairs)}
                for j, cl in enumerate(cls):
                    remaining[pair_of[cl]] += int(
                        sum(cell_chunks[cl * NW + w] for w in range(NW)))
                first = {p: True for p in range(npairs)}
                for w in range(NW):
                    base, nch = slab[(sg, w)]
                    if nch == 0:
                        continue
                    g = gpool.tile([128, nch, elem], FP16, tag="g")
                    idx_sb = ipool.tile([128, nch * 8], I16, tag="idx")
                    nc.sync.dma_start(out=idx_sb[:], in_=idx_d[:, base * 8:(base + nch) * 8])
                    for k0 in range(0, nch, CALL_CHUNKS):
                        kn = min(CALL_CHUNKS, nch - k0)
                        nc.gpsimd.dma_gather(
                            out_ap=g[:, k0:k0 + kn, :],
                            in_ap=src_dram[w * WIN: w * WIN + win_rows[w], :],
                            idxs_ap=idx_sb[:, k0 * 8:(k0 + kn) * 8],
                            num_idxs=kn * 128,
                            num_idxs_reg=kn * 128,
                            elem_size=elem,
                            queue_num=qctr[0] % NQ,
                        )
                        qctr[0] += 1
                    stiles = {}
                    ncells = {}
                    for cl in cls:
                        co = chunk_off[(cl, w)]
                        ncell = int(cell_chunks[cl * NW + w])
                        ncells[cl] = ncell
                        if ncell == 0:
                            continue
                        s = spool.tile([128, ncell, DW], FP16, tag="s")
                        nc.vector.tensor_tensor(
                            out=s[:],
                            in0=dl_sb[:, co:co + ncell].unsqueeze(-1).broadcast_to((128, ncell, DW)),
                            in1=iota[:, :ncell, :],
                            op=mybir.AluOpType.is_equal,
                        )
                        stiles[cl] = s
                    # round-robin cells from different pairs so consecutive
                    # matmuls target different PSUM banks (overlaps drain/fill)
                    rr = [cl for j, cl in enumerate(cls) if j % 2 == 0] + \
                         [cl for j, cl in enumerate(cls) if j % 2 == 1]
                    for ci in range(max(ncells.values(), default=0)):
                        for cl in rr:
                            if ci >= ncells[cl]:
                                continue
                            co = chunk_off[(cl, w)]
                            gi = co - base + ci
                            p = pair_of[cl]
                            remaining[p] -= 1
                            nc.tensor.matmul(
                                out=accs[cl][:lhs_feats, :],
                                lhsT=g[:, gi, :lhs_feats],
                                rhs=stiles[cl][:, ci, :],
                                start=first[p],
                                stop=remaining[p] == 0,
                            )
                            first[p] = False
                for cl in cls:
                    if all(cell_chunks[cl * NW + w] == 0 for w in range(NW)):
                        nc.vector.memset(accs[cl][:], 0.0)
                    evac_cell(cl, accs[cl])
                if after_group is not None:
                    after_group(sg)

        # ================= P1: u1 = A x'   (feature-major accumulate)
        def evac_p1(cl, acc):
            c0 = cl * DW
            xpp_blk = epool.tile([128, DW], FP16, tag="xpp")
            nc.sync.dma_start(out=xpp_blk[:], in_=xppT_d[:, c0:c0 + DW])
            # x1T = dinv_row * u1 + x''T      (x1 = D u1 + D^2 x'; feature-major)
            rd = repl_row(dinvrow_d, c0, DW)
            tmp = epool.tile([128, DW], F32, tag="ev32")
            nc.scalar.activation(
                out=tmp[:], in_=acc[:], func=mybir.ActivationFunctionType.Copy)
            nc.vector.tensor_tensor(
                out=tmp[:], in0=tmp[:], in1=rd[:], op=mybir.AluOpType.mult)
            x1blk = epool.tile([128, DW], FP16, tag="x1blk")
            nc.vector.tensor_tensor(
                out=x1blk[:], in0=tmp[:], in1=xpp_blk[:],
                op=mybir.AluOpType.add)
            nc.sync.dma_start(out=x1T_d[:, c0:c0 + DW], in_=x1blk[:])
            # x1' = dinv * x1 (node-major) -> cc_in1
            nt = DW // 128
            pt = tpool.tile([128, nt, 128], FP16, tag="tp16")
            for t in range(nt):
                nc.tensor.transpose(
                    out=pt[:, t, :], in_=x1blk[:, t * 128:(t + 1) * 128],
                    identity=ident[:])
            x1n = epool.tile([128, nt, 128], FP16, tag="x1n")
            t0 = cl * nt
            nc.vector.tensor_tensor(
                out=x1n[:], in0=pt[:],
                in1=dinv4w[:, t0:t0 + nt].unsqueeze(-1).broadcast_to((128, nt, 128)),
                op=mybir.AluOpType.mult)
            nc.sync.dma_start(
                out=cc_in1[c0:c0 + DW, :].rearrange("(a p) b -> p a b", p=128),
                in_=x1n[:])

        def ag_piece(cc_in, cc_out, p):
            r0 = int(piece_row_start[p])
            g0 = int(piece_base_g[p])
            nc.gpsimd.collective_compute(
                "AllGather", mybir.AluOpType.bypass,
                ins=[cc_in[r0:r0 + piece_rows[p]]],
                outs=[cc_out[g0:g0 + N_CORES * piece_rows[p]]],
                replica_groups=[list(range(N_CORES))])

        def after_group_p1(sg):
            for p, eg in enumerate(piece_end_group):
                if sg == eg:
                    ag_piece(cc_in1, cc_out1, p)

        prop(xp_d, 128, 128, evac_p1, after_group_p1)

        # ================= P2: u2 = A x1'
        def evac_p2(cl, acc):
            c0 = cl * DW
            rd = repl_row(dinvrow_d, c0, DW)
            t1 = epool.tile([128, DW], F32, tag="ev32")
            nc.scalar.activation(
                out=t1[:], in_=acc[:], func=mybir.ActivationFunctionType.Copy)
            nc.vector.tensor_tensor(
                out=t1[:], in0=t1[:], in1=rd[:], op=mybir.AluOpType.mult)
            # dinv^2 * x1 = (x1*dinv)*dinv — two multiplies, each with a single
            # PSUM operand (DVE can read only one non-scalar input from PSUM)
            x1rd = epool.tile([128, DW], FP16, tag="x1blk")
            nc.sync.dma_start(out=x1rd[:], in_=x1T_d[:, c0:c0 + DW])
            t2 = epool.tile([128, DW], F32, tag="ev32b")
            nc.vector.tensor_tensor(
                out=t2[:], in0=x1rd[:], in1=rd[:],
                op=mybir.AluOpType.mult)
            nc.vector.tensor_tensor(
                out=t2[:], in0=t2[:], in1=rd[:],
                op=mybir.AluOpType.mult)
            x2blk = epool.tile([128, DW], FP16, tag="x2blk")
            nc.vector.tensor_tensor(
                out=x2blk[:], in0=t1[:], in1=t2[:],
                op=mybir.AluOpType.add)
            nc.sync.dma_start(out=x2T_d[:, c0:c0 + DW], in_=x2blk[:])

        # ================= GEMM block (interleaved into P2's group loop)
        def gemm_block(b):
            c0 = b * NBLK
            xT_blk = epool.tile([128, NBLK], FP16, tag="xTblk")
            nc.sync.dma_start(out=xT_blk[:], in_=xT_d[:, c0:c0 + NBLK])
            x1T_blk = epool.tile([128, NBLK], FP16, tag="x1gblk")
            nc.sync.dma_start(out=x1T_blk[:], in_=x1T_d[:, c0:c0 + NBLK])
            x2T_blk = epool.tile([128, NBLK], FP16, tag="x2gblk")
            nc.sync.dma_start(out=x2T_blk[:], in_=x2T_d[:, c0:c0 + NBLK])
            h_sb = hpool.tile([128, 12, NBLK], FP16, tag="h")
            for kt in range(12):
                src = (xT_blk[:], x1T_blk[:], x2T_blk[:])[kt // 4]
                ph = tpool.tile([128, NBLK], F32, tag="tp")
                nc.tensor.matmul(
                    out=ph[:], lhsT=wcat[:, kt * 128:(kt + 1) * 128], rhs=src,
                    start=True, stop=True)
                nc.scalar.activation(
                    out=h_sb[:, kt, :], in_=ph[:],
                    func=mybir.ActivationFunctionType.Relu,
                    bias=b1w[:, kt:kt + 1])
            pz0 = tpool.tile([128, NBLK], F32, tag="tp", name="pz0")
            pz1 = tpool.tile([128, NBLK], F32, tag="tp", name="pz1")
            for kt in range(12):
                nc.tensor.matmul(
                    out=pz0[:40, :], lhsT=vt[:, kt * 80:kt * 80 + 40],
                    rhs=h_sb[:, kt, :], start=(kt == 0), stop=(kt == 11))
                nc.tensor.matmul(
                    out=pz1[:40, :], lhsT=vt[:, kt * 80 + 40:(kt + 1) * 80],
                    rhs=h_sb[:, kt, :], start=(kt == 0), stop=(kt == 11))
            # node-major z0 spill (f32), consumed by P3's evac
            z0sb = epool.tile([40, NBLK], F32, tag="z0sb")
            nc.scalar.activation(
                out=z0sb[:], in_=pz0[0:40, :], func=mybir.ActivationFunctionType.Copy)
            z0t = tpool.tile([128, 4, 40], F32, tag="tp16", name="z0t")
            for t in range(4):
                nc.tensor.transpose(
                    out=z0t[:, t, :], in_=z0sb[:, t * 128:(t + 1) * 128],
                    identity=ident32[:40, :40])
            z0n = epool.tile([128, 4, 40], F32, tag="z0n")
            nc.scalar.activation(
                out=z0n[:], in_=z0t[:], func=mybir.ActivationFunctionType.Copy)
            nc.sync.dma_start(
                out=z0n_d[c0:c0 + NBLK, :].rearrange("(a p) b -> p a b", p=128),
                in_=z0n[:])
            # node-major z1' = dinv * z1, padded to 128 cols
            z1Tt = epool.tile([40, NBLK], FP16, tag="z1Tt")
            nc.scalar.activation(
                out=z1Tt[:], in_=pz1[0:40, :], func=mybir.ActivationFunctionType.Copy)
            zt = tpool.tile([128, 4, 64], FP16, tag="tp16")
            for t in range(4):
                nc.tensor.transpose(
                    out=zt[:, t, 0:40], in_=z1Tt[:, t * 128:(t + 1) * 128],
                    identity=ident[:40, :40])
            z1n = epool.tile([128, 4, 128], FP16, tag="z1n")
            nc.vector.memset(z1n[:], 0.0)
            t4b = c0 // 128
            nc.vector.tensor_tensor(
                out=z1n[:, :, 0:40], in0=zt[:, :, 0:40],
                in1=dinv4w[:, t4b:t4b + 4].unsqueeze(-1).broadcast_to((128, 4, 40)),
                op=mybir.AluOpType.mult)
            nc.sync.dma_start(
                out=cc_in2[c0:c0 + 512, :].rearrange("(a p) b -> p a b", p=128),
                in_=z1n[:])

        def after_group_p2(sg):
            for b in range(4 * sg, min(4 * sg + 4, NB)):
                gemm_block(b)
            for p, eg in enumerate(piece_end_group):
                if sg == eg:
                    ag_piece(cc_in2, cc_out2, p)

        prop(cc_out1, 128, 128, evac_p2, after_group_p2)

        # ================= P3: u3 = A z1'  -> y -> log_softmax -> out
        # Node-major evac: y2 = dinv*(u3 + z1') + b2b since cc_in2 holds
        # z1' = dinv*z1 locally; y1 = z0 + b2a with z0 spilled node-major.
        def evac_p3(cl, acc):
            c0 = cl * DW
            nt = DW // 128
            t0 = cl * nt
            # u3 feature-major -> node-major [128, nt, 40]
            u3f = epool.tile([40, DW], F32, tag="y2")
            nc.scalar.activation(
                out=u3f[:], in_=acc[:40, :], func=mybir.ActivationFunctionType.Copy)
            u3t = tpool.tile([128, nt, 40], F32, tag="tp16", name="u3t")
            for t in range(nt):
                nc.tensor.transpose(
                    out=u3t[:, t, :], in_=u3f[:, t * 128:(t + 1) * 128],
                    identity=ident32[:40, :40])
            cc2l = epool.tile([128, nt, 40], FP16, tag="cc2l")
            nc.sync.dma_start(
                out=cc2l[:],
                in_=cc_in2[c0:c0 + DW, 0:40].rearrange("(a p) b -> p a b", p=128))
            yt = epool.tile([128, nt, 80], F32, tag="ysb2")
            nc.vector.tensor_tensor(
                out=yt[:, :, 40:80], in0=u3t[:], in1=cc2l[:],
                op=mybir.AluOpType.add)
            nc.vector.tensor_tensor(
                out=yt[:, :, 40:80], in0=yt[:, :, 40:80],
                in1=dinv4w[:, t0:t0 + nt].unsqueeze(-1).broadcast_to((128, nt, 40)),
                op=mybir.AluOpType.mult)
            nc.vector.tensor_tensor(
                out=yt[:, :, 40:80], in0=yt[:, :, 40:80],
                in1=b2brep[:].unsqueeze(1).broadcast_to((128, nt, 40)),
                op=mybir.AluOpType.add)
            # y1 = z0 + b2a (node-major load)
            z0l = epool.tile([128, nt, 40], F32, tag="z0l")
            nc.sync.dma_start(
                out=z0l[:],
                in_=z0n_d[c0:c0 + DW, :].rearrange("(a p) b -> p a b", p=128))
            nc.vector.tensor_tensor(
                out=yt[:, :, 0:40], in0=z0l[:],
                in1=b2arep[:].unsqueeze(1).broadcast_to((128, nt, 40)),
                op=mybir.AluOpType.add)
            # log_softmax over last axis
            y_sb = epool.tile([128, nt, 80], F32, tag="ysb")
            mx = epool.tile([128, nt, 1], F32, tag="mx")
            nc.vector.tensor_reduce(
                out=mx[:], in_=yt[:], axis=mybir.AxisListType.X,
                op=mybir.AluOpType.max)
            nc.vector.tensor_tensor(
                out=y_sb[:], in0=yt[:], in1=mx[:].broadcast_to((128, nt, 80)),
                op=mybir.AluOpType.subtract)
            ex = epool.tile([128, nt, 80], F32, tag="ex")
            nc.scalar.activation(
                out=ex[:], in_=y_sb[:], func=mybir.ActivationFunctionType.Exp)
            sm = epool.tile([128, nt, 1], F32, tag="sm")
            nc.vector.tensor_reduce(
                out=sm[:], in_=ex[:], axis=mybir.AxisListType.X,
                op=mybir.AluOpType.add)
            ls = epool.tile([128, nt, 1], F32, tag="ls")
            nc.scalar.activation(
                out=ls[:], in_=sm[:], func=mybir.ActivationFunctionType.Ln)
            nc.vector.tensor_tensor(
                out=y_sb[:], in0=y_sb[:], in1=ls[:].broadcast_to((128, nt, 80)),
                op=mybir.AluOpType.subtract)
            nc.sync.dma_start(
                out=out_d[c0:c0 + DW, :].rearrange("(a p) b -> p a b", p=128),
                in_=y_sb[:])

        prop(cc_out2, 128, 40, evac_p3)

    nc.compile()
    return nc


# ================================================================ entry

def kernel(x, edge_index, w1, b1, w2, b2):
    n = x.shape[0]
    dims = make_dims(n)
    shared, per_core, struct = host_prep(x, edge_index, w1, b1, w2, b2, dims)
    nc = build(dims, struct)
    in_maps = []
    for c in range(N_CORES):
        pc = per_core[c]
        in_maps.append(dict(
            xp=shared["xp"], idx=pc["idx"], dl=pc["dl"],
            xT=pc["xT"], xppT=pc["xppT"],
            dinvrow=pc["dinvrow"], dinv4w=pc["dinv4w"],
            wcat=shared["wcat"], vt=shared["vt"], b1w=shared["b1w"],
            b2arep=shared["b2arep"], b2brep=shared["b2brep"],
            iota=shared["iota"], ident=shared["ident"],
            ident32=shared["ident32"], ones32=shared["ones32"],
        ))
    res = bass_utils.run_bass_kernel_spmd(nc, in_maps, core_ids=list(range(N_CORES)))
    out = np.concatenate([res.results[c]["out"] for c in range(N_CORES)], axis=0)
    return np.ascontiguousarray(out[:n]).astype(np.float32)
